# revision 8
# baseline (speedup 1.0000x reference)
"""MoE model (router + top-2 of 8 experts, 3-layer MLP experts) on 8 trn2 cores.

Strategy: expert-parallel. The router (a tiny [4096,512]@[512,8] matmul +
softmax + top-2) runs on the host; tokens are gathered per expert on the host
and shipped to the core owning that expert. Each core runs the 3-layer MLP for
its expert on its (padded) token set in a transposed [feature, token] layout so
every layer is a plain lhsT.T @ rhs chain on the tensor engine with no on-device
transposes. The host scatters the per-expert outputs back and applies the
top-2 gate weights.

Self-contained: hardcodes all shapes from the problem spec.
"""

import numpy as np

B = 4096
D = 512
H1 = 1024
H2 = 512
C = 50
E = 8
TOP_K = 2
P = 128
N_CORES = 8

# Matmul input dtype: "f32" (exact, 4 cyc/row) or "f32r" (fast fp32, 1 cyc/row
# for moving dim >= 256, slightly reduced precision).
MM_DTYPE = "f32r"

_program_cache: dict = {}


def _token_chunks(cap: int) -> list[tuple[int, int]]:
    """Split cap (multiple of 128) into (start, size) chunks, size <= 512.

    Prefer every chunk >= 256 so float32r matmuls stay at full rate.
    """
    chunks = []
    rem = cap
    start = 0
    while rem > 0:
        if rem > 640:
            take = 512
        elif rem in (640, 512, 384, 256, 128):
            take = {640: 384, 512: 512, 384: 384, 256: 256, 128: 128}[rem]
        else:
            take = min(rem, 512)
        chunks.append((start, take))
        start += take
        rem -= take
    return chunks


def _build_program(cap: int):
    import concourse.mybir as mybir
    import concourse.tile as tile
    from concourse import bacc

    f32 = mybir.dt.float32
    mm_dt = mybir.dt.float32r if MM_DTYPE == "f32r" else mybir.dt.float32

    nc = bacc.Bacc("TRN2", target_bir_lowering=False, debug=False)
    xT_d = nc.dram_tensor("xT", [D, cap], f32, kind="ExternalInput")
    w1_d = nc.dram_tensor("w1", [D, H1], f32, kind="ExternalInput")
    b1_d = nc.dram_tensor("b1", [P, H1 // P], f32, kind="ExternalInput")
    w2_d = nc.dram_tensor("w2", [H1, H2], f32, kind="ExternalInput")
    b2_d = nc.dram_tensor("b2", [P, H2 // P], f32, kind="ExternalInput")
    w3_d = nc.dram_tensor("w3", [D, C], f32, kind="ExternalInput")
    b3_d = nc.dram_tensor("b3", [C, 1], f32, kind="ExternalInput")
    y_d = nc.dram_tensor("y", [C, cap], f32, kind="ExternalOutput")

    K1 = D // P   # 4 k-subtiles for layer 1 / layer 3
    M1 = H1 // P  # 8 output tiles for layer 1
    K2 = H1 // P  # 8 k-subtiles for layer 2
    M2 = H2 // P  # 4 output tiles for layer 2
    chunks = _token_chunks(cap)

    relu = mybir.ActivationFunctionType.Relu

    def mm(ps, lhsT, rhs, start, stop):
        nc.tensor.matmul(ps, lhsT, rhs, start=start, stop=stop)

    with tile.TileContext(nc) as tc:
        with (
            tc.tile_pool(name="sb", bufs=1) as sb,
            tc.tile_pool(name="raw", bufs=1) as raw,
            tc.tile_pool(name="ps", bufs=8, space="PSUM") as pspool,
        ):
            def load_rounded(dram_ap, shape, tag):
                """DMA f32 from DRAM, then round to the matmul dtype on DVE."""
                if mm_dt == f32:
                    t = sb.tile(shape, f32, tag=tag, name="t")
                    nc.sync.dma_start(t[:], dram_ap)
                    return t
                stage = raw.tile(shape, f32, tag="stage_" + tag, name="stage")
                nc.sync.dma_start(stage[:], dram_ap)
                t = sb.tile(shape, mm_dt, tag=tag, name="t")
                nc.vector.tensor_copy(t[:], stage)
                return t

            xTs = load_rounded(xT_d.rearrange("(k p) n -> p k n", p=P), [P, K1, cap], "xT")
            w1s = load_rounded(w1_d.rearrange("(k p) m -> p k m", p=P), [P, K1, H1], "w1")
            w2s = load_rounded(w2_d.rearrange("(k p) m -> p k m", p=P), [P, K2, H2], "w2")
            w3s = load_rounded(w3_d.rearrange("(k p) m -> p k m", p=P), [P, K1, C], "w3")
            b1s = sb.tile([P, M1], f32, tag="b1")
            nc.sync.dma_start(b1s[:], b1_d[:])
            b2s = sb.tile([P, M2], f32, tag="b2")
            nc.sync.dma_start(b2s[:], b2_d[:])
            b3s = sb.tile([C, 1], f32, tag="b3")
            nc.sync.dma_start(b3s[:], b3_d[:])

            h1s = sb.tile([P, K2, cap], mm_dt, tag="h1")
            h2s = sb.tile([P, M2, cap], mm_dt, tag="h2")
            ys = sb.tile([C, cap], f32, tag="y")

            # Layer 1: h1 = relu(W1.T @ xT + b1), [H1, cap]
            for m in range(M1):
                for n0, nt in chunks:
                    ps = pspool.tile([P, 512], f32, tag="ps", name="ps")[:, :nt]
                    for k in range(K1):
                        mm(
                            ps,
                            w1s[:, k, m * P : (m + 1) * P],
                            xTs[:, k, n0 : n0 + nt],
                            start=(k == 0),
                            stop=(k == K1 - 1),
                        )
                    nc.scalar.activation(
                        h1s[:, m, n0 : n0 + nt], ps, relu, bias=b1s[:, m : m + 1]
                    )

            # Layer 2: h2 = relu(W2.T @ h1 + b2), [H2, cap]
            for m in range(M2):
                for n0, nt in chunks:
                    ps = pspool.tile([P, 512], f32, tag="ps", name="ps")[:, :nt]
                    for k in range(K2):
                        mm(
                            ps,
                            w2s[:, k, m * P : (m + 1) * P],
                            h1s[:, k, n0 : n0 + nt],
                            start=(k == 0),
                            stop=(k == K2 - 1),
                        )
                    nc.vector.tensor_scalar(
                        h2s[:, m, n0 : n0 + nt],
                        ps,
                        b2s[:, m : m + 1],
                        0.0,
                        mybir.AluOpType.add,
                        mybir.AluOpType.max,
                    )

            # Layer 3: y = W3.T @ h2 + b3, [C, cap]
            for n0, nt in chunks:
                ps = pspool.tile([P, 512], f32, tag="ps", name="ps")[:C, :nt]
                for k in range(K1):
                    mm(
                        ps,
                        w3s[:, k, :],
                        h2s[:, k, n0 : n0 + nt],
                        start=(k == 0),
                        stop=(k == K1 - 1),
                    )
                nc.vector.tensor_scalar_add(ys[:, n0 : n0 + nt], ps, b3s[:, :1])

            nc.sync.dma_start(y_d[:], ys[:])

    nc.compile()
    return nc


def _get_program(cap: int):
    if cap not in _program_cache:
        _program_cache[cap] = _build_program(cap)
    return _program_cache[cap]


def kernel(x, Wr, br, W1, b1, W2, b2, W3, b3, _run_opts=None):
    from concourse import bass_utils

    x = np.ascontiguousarray(np.asarray(x, dtype=np.float32))
    Wr = np.asarray(Wr, dtype=np.float32)
    br = np.asarray(br, dtype=np.float32)
    W1 = np.asarray(W1, dtype=np.float32)
    b1 = np.asarray(b1, dtype=np.float32)
    W2 = np.asarray(W2, dtype=np.float32)
    b2 = np.asarray(b2, dtype=np.float32)
    W3 = np.asarray(W3, dtype=np.float32)
    b3 = np.asarray(b3, dtype=np.float32)

    # ---- Router on host (tiny): probs = softmax(x @ Wr + br), top-2 ----
    logits = x @ Wr + br
    m = logits.max(axis=1, keepdims=True)
    ex = np.exp(logits - m)
    probs = ex / ex.sum(axis=1, keepdims=True)
    # stable argsort matches jax.lax.top_k tie-breaking (lowest index first)
    top2 = np.argsort(-probs, axis=1, kind="stable")[:, :TOP_K]

    tok_ids = []
    gates = []
    for e in range(E):
        te = np.nonzero((top2 == e).any(axis=1))[0]
        tok_ids.append(te)
        gates.append(probs[te, e])
    counts = [len(t) for t in tok_ids]
    cap = max(512, -(-max(counts) // P) * P)

    nc = _get_program(cap)

    in_maps = []
    for e in range(E):
        te = tok_ids[e]
        xe = np.zeros((cap, D), dtype=np.float32)
        xe[: counts[e]] = x[te]
        in_maps.append(
            {
                "xT": np.ascontiguousarray(xe.T),
                "w1": np.ascontiguousarray(W1[e]),
                "b1": np.ascontiguousarray(b1[e].reshape(H1 // P, P).T),
                "w2": np.ascontiguousarray(W2[e]),
                "b2": np.ascontiguousarray(b2[e].reshape(H2 // P, P).T),
                "w3": np.ascontiguousarray(W3[e]),
                "b3": np.ascontiguousarray(b3[e].reshape(C, 1)),
            }
        )

    run_opts = dict(_run_opts or {})
    res = bass_utils.run_bass_kernel_spmd(
        nc, in_maps, core_ids=list(range(N_CORES)), **run_opts
    )

    out = np.zeros((B, C), dtype=np.float32)
    for e in range(E):
        ye = res.results[e]["y"][:, : counts[e]].T  # [count, C]
        out[tok_ids[e]] += gates[e][:, None] * ye
    out *= 1.0 / TOP_K

    if _run_opts is not None:
        return (out, probs), res
    return out, probs


# revision 11
# speedup vs baseline: 1.1864x; 1.1864x over previous
"""MoE model (router + top-2 of 8 experts, 3-layer MLP experts) on 8 trn2 cores.

Strategy: expert-parallel. The router (a tiny [4096,512]@[512,8] matmul +
softmax + top-2) runs on the host; tokens are gathered per expert on the host
and shipped to the core owning that expert. Each core runs the 3-layer MLP for
its expert on its token set in a transposed [feature, token] layout so every
layer is a plain lhsT.T @ rhs chain on the tensor engine with no on-device
transposes. The host scatters the per-expert outputs back and applies the
top-2 gate weights.

Matmuls run in float32r (fast fp32 mode, ~1 cycle/row for moving dim >= 256).
The PE rounds raw f32 inputs internally, so tensors are DMA'd straight into
float32r SBUF tiles (verified bit-identical to an explicit cast on HW).

Self-contained: hardcodes all shapes from the problem spec.
"""

import math

import numpy as np

B = 4096
D = 512
H1 = 1024
H2 = 512
C = 50
E = 8
TOP_K = 2
P = 128
N_CORES = 8

# "f32r" (fast fp32, ~1 cyc/row, rel err ~2e-4) or "f32" (exact, 4 cyc/row).
MM_DTYPE = "f32r"

_program_cache: dict = {}


def _token_chunks(cap: int) -> list[tuple[int, int]]:
    """Split cap into equal-ish (start, size) chunks of at most 512 columns.

    Equal splitting keeps every chunk >= 256 whenever cap >= 512, so float32r
    matmuls stay at full rate. Sizes are even (f32r ISA requirement); cap must
    be even."""
    assert cap % 2 == 0
    parts = max(1, -(-cap // 512))
    half = cap // 2
    base, rem = divmod(half, parts)
    sizes = [2 * (base + 1)] * rem + [2 * base] * (parts - rem)
    out, start = [], 0
    for s in sizes:
        out.append((start, s))
        start += s
    return out


def _build_program(cap: int):
    import concourse.mybir as mybir
    import concourse.tile as tile
    from concourse import bacc

    f32 = mybir.dt.float32
    mm_dt = mybir.dt.float32r if MM_DTYPE == "f32r" else mybir.dt.float32

    nc = bacc.Bacc("TRN2", target_bir_lowering=False, debug=False)
    xT_d = nc.dram_tensor("xT", [D, cap], mm_dt, kind="ExternalInput")
    w1_d = nc.dram_tensor("w1", [D, H1], mm_dt, kind="ExternalInput")
    w2_d = nc.dram_tensor("w2", [H1, H2], mm_dt, kind="ExternalInput")
    w3_d = nc.dram_tensor("w3p", [P, (D // P) * C], mm_dt, kind="ExternalInput")
    b1_d = nc.dram_tensor("b1", [P, H1 // P], f32, kind="ExternalInput")
    b2_d = nc.dram_tensor("b2", [P, H2 // P], f32, kind="ExternalInput")
    b3_d = nc.dram_tensor("b3", [C, 1], f32, kind="ExternalInput")
    y_d = nc.dram_tensor("y", [C, cap], f32, kind="ExternalOutput")

    K1 = D // P   # 4 k-subtiles for layer 1 / layer 3
    M1 = H1 // P  # 8 output tiles for layer 1
    K2 = H1 // P  # 8 k-subtiles for layer 2
    M2 = H2 // P  # 4 output tiles for layer 2
    chunks = _token_chunks(cap)

    relu = mybir.ActivationFunctionType.Relu

    with tile.TileContext(nc) as tc:
        with (
            tc.tile_pool(name="sb", bufs=1) as sb,
            tc.tile_pool(name="ps", bufs=8, space="PSUM") as pspool,
        ):
            xTs = sb.tile([P, K1, cap], mm_dt, tag="xT")
            w1s = sb.tile([P, K1, H1], mm_dt, tag="w1")
            w2s = sb.tile([P, K2, H2], mm_dt, tag="w2")
            w3s = sb.tile([P, K1, C], mm_dt, tag="w3")
            b1s = sb.tile([P, M1], f32, tag="b1")
            b2s = sb.tile([P, M2], f32, tag="b2")
            b3s = sb.tile([C, 1], f32, tag="b3")
            h1s = sb.tile([P, K2, cap], mm_dt, tag="h1")
            h2s = sb.tile([P, M2, cap], mm_dt, tag="h2")
            ys = sb.tile([C, cap], f32, tag="y")

            # DMAs in order of first use; per-k/per-chunk pieces so compute
            # can start as soon as the first pieces land.
            n0, c0 = chunks[0]
            for k in range(K1):
                nc.sync.dma_start(
                    xTs[:, k, n0 : n0 + c0], xT_d[k * P : (k + 1) * P, n0 : n0 + c0]
                )
            nc.sync.dma_start(b1s[:], b1_d[:])
            nc.sync.dma_start(b2s[:], b2_d[:])
            nc.sync.dma_start(b3s[:], b3_d[:])
            for k in range(K1):
                nc.sync.dma_start(w1s[:, k, :], w1_d[k * P : (k + 1) * P, :])
            for k in range(K2):
                nc.sync.dma_start(w2s[:, k, :], w2_d[k * P : (k + 1) * P, :])
            nc.sync.dma_start(
                w3s[:].rearrange("p k c -> p (k c)"), w3_d[:]
            )
            for n0, nt in chunks[1:]:
                for k in range(K1):
                    nc.sync.dma_start(
                        xTs[:, k, n0 : n0 + nt], xT_d[k * P : (k + 1) * P, n0 : n0 + nt]
                    )

            for n0, nt in chunks:
                # Layer 1: h1 = relu(W1.T @ xT + b1)
                for m in range(M1):
                    ps = pspool.tile([P, 512], f32, tag="ps", name="ps")[:, :nt]
                    for k in range(K1):
                        nc.tensor.matmul(
                            ps,
                            w1s[:, k, m * P : (m + 1) * P],
                            xTs[:, k, n0 : n0 + nt],
                            start=(k == 0),
                            stop=(k == K1 - 1),
                        )
                    nc.scalar.activation(
                        h1s[:, m, n0 : n0 + nt], ps, relu, bias=b1s[:, m : m + 1]
                    )
                # Layer 2: h2 = relu(W2.T @ h1 + b2)
                for m in range(M2):
                    ps = pspool.tile([P, 512], f32, tag="ps", name="ps")[:, :nt]
                    for k in range(K2):
                        nc.tensor.matmul(
                            ps,
                            w2s[:, k, m * P : (m + 1) * P],
                            h1s[:, k, n0 : n0 + nt],
                            start=(k == 0),
                            stop=(k == K2 - 1),
                        )
                    nc.vector.tensor_scalar(
                        h2s[:, m, n0 : n0 + nt],
                        ps,
                        b2s[:, m : m + 1],
                        0.0,
                        mybir.AluOpType.add,
                        mybir.AluOpType.max,
                    )
                # Layer 3: y = W3.T @ h2 + b3
                ps = pspool.tile([P, 512], f32, tag="ps", name="ps")[:C, :nt]
                for k in range(K1):
                    nc.tensor.matmul(
                        ps,
                        w3s[:, k, :],
                        h2s[:, k, n0 : n0 + nt],
                        start=(k == 0),
                        stop=(k == K1 - 1),
                    )
                nc.vector.tensor_scalar_add(ys[:, n0 : n0 + nt], ps, b3s[:, :1])
                nc.sync.dma_start(y_d[:, n0 : n0 + nt], ys[:, n0 : n0 + nt])

    nc.compile()
    return nc


def _get_program(cap: int):
    if cap not in _program_cache:
        _program_cache[cap] = _build_program(cap)
    return _program_cache[cap]


def kernel(x, Wr, br, W1, b1, W2, b2, W3, b3, _run_opts=None):
    from concourse import bass_utils

    x = np.ascontiguousarray(np.asarray(x, dtype=np.float32))
    Wr = np.asarray(Wr, dtype=np.float32)
    br = np.asarray(br, dtype=np.float32)
    W1 = np.asarray(W1, dtype=np.float32)
    b1 = np.asarray(b1, dtype=np.float32)
    W2 = np.asarray(W2, dtype=np.float32)
    b2 = np.asarray(b2, dtype=np.float32)
    W3 = np.asarray(W3, dtype=np.float32)
    b3 = np.asarray(b3, dtype=np.float32)

    # ---- Router on host (tiny): probs = softmax(x @ Wr + br), top-2 ----
    logits = x @ Wr + br
    m = logits.max(axis=1, keepdims=True)
    ex = np.exp(logits - m)
    probs = ex / ex.sum(axis=1, keepdims=True)
    # stable argsort matches jax.lax.top_k tie-breaking (lowest index first)
    top2 = np.argsort(-probs, axis=1, kind="stable")[:, :TOP_K]

    tok_ids = []
    gates = []
    for e in range(E):
        te = np.nonzero((top2 == e).any(axis=1))[0]
        tok_ids.append(te)
        gates.append(probs[te, e])
    counts = [len(t) for t in tok_ids]
    cap = max(64, max(counts))
    cap += cap % 2

    nc = _get_program(cap)

    in_maps = []
    for e in range(E):
        te = tok_ids[e]
        xe = np.zeros((cap, D), dtype=np.float32)
        xe[: counts[e]] = x[te]
        in_maps.append(
            {
                "xT": np.ascontiguousarray(xe.T),
                "w1": W1[e],
                "w2": W2[e],
                "w3p": np.ascontiguousarray(
                    W3[e].reshape(D // P, P, C).transpose(1, 0, 2).reshape(P, -1)
                ),
                "b1": np.ascontiguousarray(b1[e].reshape(H1 // P, P).T),
                "b2": np.ascontiguousarray(b2[e].reshape(H2 // P, P).T),
                "b3": np.ascontiguousarray(b3[e].reshape(C, 1)),
            }
        )

    run_opts = dict(_run_opts or {})
    res = bass_utils.run_bass_kernel_spmd(
        nc, in_maps, core_ids=list(range(N_CORES)), **run_opts
    )

    out = np.zeros((B, C), dtype=np.float32)
    for e in range(E):
        ye = res.results[e]["y"][:, : counts[e]].T  # [count, C]
        out[tok_ids[e]] += gates[e][:, None] * ye
    out *= 1.0 / TOP_K

    if _run_opts is not None:
        return (out, probs), res
    return out, probs
